# revision 33
# baseline (speedup 1.0000x reference)
"""Trainium2 Bass kernel for a 6-layer GPT forward pass (B=4, T=1024, D=512,
H=8, HS=64, FF=2048, V=50257) on 8 NeuronCores.

Strategy: Megatron-style tensor+sequence parallel PAIRS over batch.
  - Cores c and c+4 form a pair working on batch c%4.
  - The residual stream x, LayerNorms, and the MLP are sharded by TOKENS
    (core c owns tokens [512*(c//4), 512*(c//4)+512)).
  - QKV projections, attention, and the attn-out projection are sharded by
    HEADS (4 heads per core, host slices Wq/Wk/Wv/Wproj — data, not code,
    so the single SPMD NEFF stays uniform across cores).
  - Per layer: AllGather of the LN1 output h (bf16) so QKV sees all tokens,
    and a ReduceScatter(add) of the attn-projection partial update (fp32)
    back to token shards.  MLP needs no communication.
  - LM head: each core computes logits for ITS 512 tokens x the FULL vocab
    (bf16 out, host upcasts + reassembles).

All activations stay TRANSPOSED [D, tokens] so every matmul contracts on
partitions; per-token row stats are broadcast via the idle GPSIMD engine.
"""

import numpy as np
import ml_dtypes

import concourse.bass as bass
import concourse.bacc as bacc
import concourse.mybir as mybir
from concourse.bass import ts, ds
from concourse.tile import TileContext
from concourse.bass_utils import run_bass_kernel_spmd

# Prefer the combined ln+exp table set so Ln/Exp activations don't ping-pong
# ACT_TABLE_LOADs between per-function home sets (~1.3us per switch).
import concourse.hw_specs as _hw_specs
import concourse.bacc as _bacc_mod

_orig_get_tables = _hw_specs.get_activation_tables


def _tables_combined_first(module_arch):
    tabs = _orig_get_tables(module_arch)
    pref = "natural_log_exp_and_others"
    if pref not in tabs:
        return tabs
    excl = {AF.Exp, AF.Ln}
    return {k: (v if k == pref else (v - excl)) for k, v in tabs.items()}


AF = mybir.ActivationFunctionType
_bacc_mod.get_activation_tables = _tables_combined_first
F32 = mybir.dt.float32
BF16 = mybir.dt.bfloat16

P = 128
B, T, D, H, HS, FF, L, V = 4, 1024, 512, 8, 64, 2048, 6, 50257
DC = D // P            # 4 d-chunks
FC = FF // P           # 16 ff-chunks
NT = T // P            # 8 token chunks of 128
NJ = T // 512          # 2 token chunks of 512
TM = 512               # tokens owned per core
NTM = TM // P          # 4 owned token chunks
HL = 4                 # heads per core
QC = HL * HS // P      # 2 chunks of my-head q/k dims
NV = 25216
VPAD = 2 * NV          # 50432 padded vocab (full, per core)
EPS = 1e-5
N_CORES = 8
GROUPS = [[0, 4], [1, 5], [2, 6], [3, 7]]

bf16_np = ml_dtypes.bfloat16


# --------------------------------------------------------------------------
# device program
# --------------------------------------------------------------------------

def build_nc(n_layers=L, debug=False):
    nc = bacc.Bacc()

    # ---------------- I/O ----------------
    x0_d = nc.dram_tensor("x0", [D, TM], F32, kind="ExternalInput")
    wq_d = nc.dram_tensor("wq", [n_layers, D, HL * HS], BF16, kind="ExternalInput")
    wk_d = nc.dram_tensor("wk", [n_layers, D, HL * HS], BF16, kind="ExternalInput")
    wv_d = nc.dram_tensor("wv", [n_layers, D, HL * HS], BF16, kind="ExternalInput")
    wp_d = nc.dram_tensor("wp", [n_layers, HL * HS, D], BF16, kind="ExternalInput")
    w1_d = nc.dram_tensor("w1", [n_layers, D, FF], BF16, kind="ExternalInput")
    w2_d = nc.dram_tensor("w2", [n_layers, FF, D], BF16, kind="ExternalInput")
    wlm_d = nc.dram_tensor("wlm", [D, VPAD], BF16, kind="ExternalInput")
    out_d = nc.dram_tensor("logits", [TM, VPAD], BF16, kind="ExternalOutput")

    # ---------------- constants ----------------
    # causal masks for transposed scores [t_k (partition), t_q (free)]:
    # block (r) valid iff t_k_local + 128*r <= t_q_local (within a 512 chunk)
    mask_np = np.zeros((P, 2, 1024), dtype=bf16_np)
    for pair in range(2):
        for half in range(2):
            r = 2 * pair + half
            tk = np.arange(P)[:, None] + 128 * r
            tq = np.arange(512)[None, :]
            mask_np[:, pair, half * 512:(half + 1) * 512] = \
                (tk <= tq).astype(bf16_np)
    mask_c = nc.inline_tensor(mask_np, name="cmask")
    ones_f32_c = nc.inline_tensor(np.ones((P, 1), np.float32), name="ones_f")
    ones_bf_c = nc.inline_tensor(np.ones((P, 1), bf16_np), name="ones_b")

    with TileContext(nc) as tc:
        with tc.tile_pool(name="persist", bufs=1) as persist:
            # ---- persistent tiles (token-sharded unless noted) ----
            x_sb = persist.tile([P, DC, TM], F32)          # my residual x^T
            hm_sb = persist.tile([P, DC, TM], BF16)        # my LN output
            hf_sb = persist.tile([P, DC, T], BF16)         # gathered h (full T)
            q_sb = persist.tile([P, QC, T], BF16)          # my-heads Q^T
            k_sb = persist.tile([P, QC, T], BF16)          # my-heads K^T
            v_sb = persist.tile([P, NT, HL, HS + 1], BF16)
            ac_sb = persist.tile([P, QC, T], BF16)         # my-heads attn^T
            pu_sb = persist.tile([P, DC, T], BF16)         # proj partial
            pr_sb = persist.tile([P, DC, TM], BF16)        # reduced update
            mid_sb = persist.tile([P, FC, TM], BF16)       # MLP mid^T
            mask_sb = persist.tile([P, 2, 1024], BF16)
            ones_f = persist.tile([P, 1], F32)
            ones_b = persist.tile([P, 1], BF16)

            nc.gpsimd.dma_start(mask_sb[:], mask_c[:])
            nc.gpsimd.dma_start(ones_f[:], ones_f32_c[:])
            nc.gpsimd.dma_start(ones_b[:], ones_bf_c[:])
            nc.gpsimd.dma_start(
                x_sb[:], x0_d[:].rearrange("(c p) t -> p c t", p=P))

            nc.vector.memset(v_sb[:, :, :, HS], 1.0)

            with (
                tc.tile_pool(name="wqkv", bufs=1) as wqkv_pool,
                tc.tile_pool(name="w1p", bufs=1) as w1_pool,
                tc.tile_pool(name="w2p", bufs=1) as w2_pool,
                tc.tile_pool(name="tmp", bufs=2) as tmp_pool,
                tc.tile_pool(name="wei", bufs=4) as wei_pool,
                tc.tile_pool(name="rows", bufs=2) as row_pool,
                tc.tile_pool(name="dramcc", bufs=2, space="DRAM") as dram_pool,
                tc.tile_pool(name="ps_wide", bufs=2, space="PSUM") as ps_wide,
                tc.tile_pool(name="ps_att", bufs=4, space="PSUM") as ps_att,
            ):
                # ---- helpers ----
                def layer_norm(src_sb, dst_sb, tok0=0, ntok=TM, w=512):
                    """src [P, DC, *] f32 -> dst bf16; LN over D (my tokens
                    [tok0, tok0+ntok)).  gamma==1/beta==0 (host-asserted)."""
                    for j in range(ntok // w):
                        sl = ds(tok0 + j * w, w)
                        xsq = tmp_pool.tile([P, DC, w], BF16, tag="xsq")
                        for c in range(DC):
                            nc.scalar.activation(
                                xsq[:, c, :], src_sb[:, c, sl], AF.Square)
                        st_s = ps_att.tile([1, w], F32, tag="att")
                        st_q = ps_att.tile([1, w], F32, tag="att")
                        for c in range(DC):
                            nc.tensor.matmul(st_s[:], ones_f[:],
                                             src_sb[:, c, sl],
                                             start=(c == 0), stop=(c == DC - 1))
                            nc.tensor.matmul(st_q[:], ones_b[:], xsq[:, c, :],
                                             start=(c == 0), stop=(c == DC - 1))
                        r_mun = row_pool.tile([1, w], F32, tag="r_mun")
                        r_msq = row_pool.tile([1, w], F32, tag="r_msq")
                        r_var = row_pool.tile([1, w], F32, tag="r_var")
                        rstd_b = row_pool.tile([1, w], BF16, tag="rstd_b")
                        nmr_b = row_pool.tile([1, w], BF16, tag="nmr_b")
                        nc.vector.tensor_scalar_mul(r_mun[:], st_s[:], -1.0 / D)
                        nc.vector.tensor_scalar_mul(r_msq[:], st_q[:], 1.0 / D)
                        nc.vector.tensor_mul(r_var[:], r_mun[:], r_mun[:])
                        nc.vector.tensor_sub(r_var[:], r_msq[:], r_var[:])
                        nc.vector.tensor_scalar_add(r_var[:], r_var[:], EPS)
                        nc.scalar.activation(r_var[:], r_var[:], AF.Ln)
                        nc.scalar.activation(rstd_b[:], r_var[:], AF.Exp,
                                             scale=-0.5)
                        nc.vector.tensor_mul(nmr_b[:], r_mun[:], rstd_b[:])
                        bc = tmp_pool.tile([P, 2, w], BF16, tag="lnbc")
                        nc.gpsimd.partition_broadcast(bc[:, 0, :], rstd_b[:])
                        nc.gpsimd.partition_broadcast(bc[:, 1, :], nmr_b[:])
                        for c in range(DC):
                            tmp = tmp_pool.tile([P, w], F32, tag="lnt")
                            nc.vector.tensor_mul(tmp[:], src_sb[:, c, sl],
                                                 bc[:, 0, :])
                            nc.vector.tensor_add(dst_sb[:, c, sl], tmp[:],
                                                 bc[:, 1, :])

                def linear_T(w_sb, src_sb, M_chunks, K_chunks, tok0, ntok, w,
                             evict):
                    for j in range(ntok // w):
                        base = tok0 + j * w
                        for m in range(M_chunks):
                            pt = ps_wide.tile([P, w], F32, tag="wide")
                            for c in range(K_chunks):
                                nc.tensor.matmul(pt[:], w_sb[:, c, ts(m, P)],
                                                 src_sb[:, c, ds(base, w)],
                                                 start=(c == 0),
                                                 stop=(c == K_chunks - 1))
                            evict(pt, m, base, w)

                # ================= transformer layers =================
                for l in range(n_layers):
                    wq_sb = wqkv_pool.tile([P, DC, HL * HS], BF16, tag="wq")
                    wk_sb = wqkv_pool.tile([P, DC, HL * HS], BF16, tag="wk")
                    wv_sb = wqkv_pool.tile([P, DC, HL * HS], BF16, tag="wv")
                    wp_sb = wqkv_pool.tile([P, QC, D], BF16, tag="wp")
                    w1_sb = w1_pool.tile([P, DC, FF], BF16, tag="w1")
                    w2_sb = w2_pool.tile([P, FC, D], BF16, tag="w2")
                    nc.gpsimd.dma_start(
                        wq_sb[:], wq_d[l].rearrange("(c p) m -> p c m", p=P))
                    nc.gpsimd.dma_start(
                        wk_sb[:], wk_d[l].rearrange("(c p) m -> p c m", p=P))
                    nc.gpsimd.dma_start(
                        wv_sb[:], wv_d[l].rearrange("(c p) m -> p c m", p=P))
                    nc.gpsimd.dma_start(
                        wp_sb[:], wp_d[l].rearrange("(c p) m -> p c m", p=P))
                    nc.gpsimd.dma_start(
                        w1_sb[:], w1_d[l].rearrange("(c p) m -> p c m", p=P))
                    nc.gpsimd.dma_start(
                        w2_sb[:], w2_d[l].rearrange("(c p) m -> p c m", p=P))

                    # -- LN1 (my tokens) --
                    layer_norm(x_sb, hm_sb)

                    # -- AllGather h in two half-token collectives so the
                    # second hides behind QKV matmuls on the first set.
                    # Set s covers absolute tokens {g*512+s*256..+256}.
                    for s in range(2):
                        ag_in = dram_pool.tile([P, DC, 256], BF16,
                                               tag=f"ag_in{s}")
                        ag_out = dram_pool.tile([2, P, DC, 256], BF16,
                                                tag=f"ag_out{s}")
                        nc.gpsimd.dma_start(
                            ag_in[:], hm_sb[:, :, ds(s * 256, 256)])
                        nc.gpsimd.collective_compute(
                            "AllGather", mybir.AluOpType.bypass,
                            replica_groups=GROUPS,
                            ins=[ag_in.opt()], outs=[ag_out.opt()])
                        for g in range(2):
                            nc.gpsimd.dma_start(
                                hf_sb[:, :, ds(g * 512 + s * 256, 256)],
                                ag_out[g])

                    # -- my-heads Q^T, K^T over all tokens (set-ordered) --
                    hf4 = hf_sb[:].rearrange("p c (g s n) -> p c g s n",
                                             g=2, s=2)
                    for s in range(2):
                        for dst, w_sb in ((q_sb, wq_sb), (k_sb, wk_sb)):
                            for m in range(QC):
                                pt = ps_wide.tile([P, 512], F32, tag="wide")
                                for c in range(DC):
                                    nc.tensor.matmul(
                                        pt[:], w_sb[:, c, ts(m, P)],
                                        hf4[:, c, :, s, :],
                                        start=(c == 0), stop=(c == DC - 1))
                                nc.vector.tensor_copy(
                                    dst[:, m, :].rearrange(
                                        "p (g s n) -> p g s n",
                                        g=2, s=2)[:, :, s, :],
                                    pt[:].rearrange("p (g n) -> p g n", g=2))

                        # -- V natural for this set's token chunks --
                        for tchunk in (s * 2, s * 2 + 1, 4 + s * 2,
                                       5 + s * 2):
                            pt = ps_wide.tile([P, 512], F32, tag="wide")
                            for c in range(DC):
                                nc.tensor.matmul(pt[:, :HL * HS],
                                                 hf_sb[:, c, ts(tchunk, P)],
                                                 wv_sb[:, c, :],
                                                 start=(c == 0),
                                                 stop=(c == DC - 1))
                            nc.vector.tensor_copy(
                                v_sb[:, tchunk, :, 0:HS],
                                pt[:, :HL * HS].rearrange(
                                    "p (h s) -> p h s", h=HL))

                    # -- attention: my 2 head-pairs, all tq --
                    for hp in range(HL // 2):
                        h0, h1 = 2 * hp, 2 * hp + 1
                        for j in range(NJ):
                            kmax = 4 * j + 4
                            pa0 = ps_att.tile([HS + 1, 512], F32, tag="att")
                            pa1 = ps_att.tile([HS + 1, 512], F32, tag="att")
                            for kp in range(kmax // 2):
                                kk0 = 2 * kp
                                r = kk0 - 4 * j
                                weis = []
                                for idx in (0, 1):
                                    off = 64 * idx
                                    pscr = ps_wide.tile([P, 1024], F32,
                                                        tag="wide")
                                    for half in (0, 1):
                                        nc.tensor.matmul(
                                            pscr[:, ds(half * 512, 512)],
                                            k_sb[off:off + HS, hp,
                                                 ts(kk0 + half, P)],
                                            q_sb[off:off + HS, hp,
                                                 ts(j, 512)],
                                            start=True, stop=True)
                                    wei = wei_pool.tile([P, 1024], BF16,
                                                        tag="wei")
                                    nc.scalar.activation(wei[:], pscr[:],
                                                         AF.Exp)
                                    if r >= 0:
                                        nc.vector.tensor_mul(
                                            wei[:], wei[:],
                                            mask_sb[:, r // 2, :])
                                    weis.append(wei)
                                for half in (0, 1):
                                    kk = kk0 + half
                                    hs_sl = ds(half * 512, 512)
                                    nc.tensor.matmul(
                                        pa0[:], v_sb[:, kk, h0, :],
                                        weis[0][:, hs_sl],
                                        start=(kk == 0),
                                        stop=(kk == kmax - 1))
                                    nc.tensor.matmul(
                                        pa1[:], v_sb[:, kk, h1, :],
                                        weis[1][:, hs_sl],
                                        start=(kk == 0),
                                        stop=(kk == kmax - 1))
                            for idx, pa in enumerate((pa0, pa1)):
                                off = 64 * idx
                                l_row = row_pool.tile([1, 512], BF16,
                                                      tag="l_row")
                                nc.vector.tensor_copy(l_row[:],
                                                      pa[HS:HS + 1, :])
                                lb = tmp_pool.tile([64, 512], BF16,
                                                   tag="lb")
                                nc.gpsimd.partition_broadcast(lb[:], l_row[:])
                                rinv = tmp_pool.tile([64, 512], F32,
                                                     tag="rinv")
                                nc.scalar.activation(rinv[:], lb[:], AF.Ln)
                                nc.scalar.activation(rinv[:], rinv[:], AF.Exp,
                                                     scale=-1.0)
                                nc.vector.tensor_mul(
                                    ac_sb[off:off + HS, hp, ts(j, 512)],
                                    pa[0:HS, :], rinv[:])

                    # -- proj partial (my heads) in two token-sets, each
                    # ReduceScattered while the other set's MLP runs --
                    ac4 = ac_sb[:].rearrange("p c (g s n) -> p c g s n",
                                             g=2, s=2)
                    pu4 = pu_sb[:].rearrange("p c (g s n) -> p c g s n",
                                             g=2, s=2)
                    for s in range(2):
                        for m in range(DC):
                            pt = ps_wide.tile([P, 512], F32, tag="wide")
                            for c in range(QC):
                                nc.tensor.matmul(pt[:], wp_sb[:, c, ts(m, P)],
                                                 ac4[:, c, :, s, :],
                                                 start=(c == 0),
                                                 stop=(c == QC - 1))
                            nc.vector.tensor_copy(
                                pu4[:, m, :, s, :],
                                pt[:].rearrange("p (g n) -> p g n", g=2))
                        rs_in = dram_pool.tile([2, P, DC, 256], BF16,
                                               tag=f"rs_in{s}")
                        rs_out = dram_pool.tile([P, DC, 256], BF16,
                                                tag=f"rs_out{s}")
                        for g in range(2):
                            nc.gpsimd.dma_start(rs_in[g], pu4[:, :, g, s, :])
                        nc.gpsimd.collective_compute(
                            "ReduceScatter", mybir.AluOpType.add,
                            replica_groups=GROUPS,
                            ins=[rs_in.opt()], outs=[rs_out.opt()])
                        nc.gpsimd.dma_start(pr_sb[:, :, ds(s * 256, 256)],
                                            rs_out[:])

                    def evict_mid(pt, m, base, w):
                        nc.scalar.activation(mid_sb[:, m, ds(base, w)], pt[:],
                                             AF.Relu)

                    def evict_resid(pt, m, base, w):
                        nc.vector.tensor_add(x_sb[:, m, ds(base, w)],
                                             x_sb[:, m, ds(base, w)], pt[:])

                    # -- LN2 + MLP per 256-token set (set 0 overlaps RS 1) --
                    for s in range(2):
                        t0 = s * 256
                        for c in range(DC):
                            prf = tmp_pool.tile([P, 256], F32, tag="prf")
                            nc.vector.tensor_copy(prf[:],
                                                  pr_sb[:, c, ds(t0, 256)])
                            nc.vector.tensor_add(x_sb[:, c, ds(t0, 256)],
                                                 x_sb[:, c, ds(t0, 256)],
                                                 prf[:])
                        layer_norm(x_sb, hm_sb, tok0=t0, ntok=256, w=256)
                        linear_T(w1_sb, hm_sb, FC, DC, t0, 256, 256,
                                 evict_mid)
                        linear_T(w2_sb, mid_sb, DC, FC, t0, 256, 256,
                                 evict_resid)

                # ================= final LN =================
                layer_norm(x_sb, hm_sb)

            # ================= logits (my tokens x full vocab) ========
            with (
                tc.tile_pool(name="wlmp", bufs=2) as wlm_pool,
                tc.tile_pool(name="stage", bufs=3) as stage_pool,
                tc.tile_pool(name="ps_log", bufs=6, space="PSUM") as ps_log,
            ):
                GW = 6 * 512
                n_groups = (VPAD + GW - 1) // GW
                for g in range(n_groups):
                    g0 = g * GW
                    gw = min(GW, VPAD - g0)
                    wlm_sb = wlm_pool.tile([P, DC, GW], BF16, tag="wlm")
                    nc.gpsimd.dma_start(
                        wlm_sb[:, :, :gw],
                        wlm_d[:][:, g0:g0 + gw].rearrange(
                            "(c p) n -> p c n", p=P))
                    n_sub = (gw + 511) // 512
                    for m in range(NTM):
                        st = stage_pool.tile([P, GW], BF16, tag="stage")
                        for n in range(n_sub):
                            nw = min(512, gw - n * 512)
                            pt = ps_log.tile([P, 512], F32, tag="log")
                            for c in range(DC):
                                nc.tensor.matmul(
                                    pt[:, :nw],
                                    hm_sb[:, c, ts(m, P)],
                                    wlm_sb[:, c, ds(n * 512, nw)],
                                    start=(c == 0), stop=(c == DC - 1))
                            if n % 2 == 0:
                                nc.scalar.copy(st[:, ds(n * 512, nw)],
                                               pt[:, :nw])
                            else:
                                nc.vector.tensor_copy(st[:, ds(n * 512, nw)],
                                                      pt[:, :nw])
                        nc.sync.dma_start(out_d[:][ts(m, P), g0:g0 + gw],
                                          st[:, :gw])

    nc.compile()
    return nc


# --------------------------------------------------------------------------
# host side
# --------------------------------------------------------------------------

_NC_CACHE = {}


def _get_nc(n_layers=L, debug=False):
    key = (n_layers, debug)
    if key not in _NC_CACHE:
        _NC_CACHE[key] = build_nc(n_layers, debug)
    return _NC_CACHE[key]


def _prep_in_maps(index, tok_emb, pos_emb, Wq, Wk, Wv, Wproj, bproj,
                  ln1_g, ln1_b, ln2_g, ln2_b, W1, b1, W2, b2,
                  lnf_g, lnf_b, Wlm, n_layers=L):
    f32 = np.float32
    idx = np.asarray(index)
    tok = np.asarray(tok_emb, f32)
    pos = np.asarray(pos_emb, f32)
    x0 = tok[idx] + pos[None, :T]                       # [B, T, D]
    x0_t = np.ascontiguousarray(x0.transpose(0, 2, 1))  # [B, D, T]

    def to_bf(a):
        return np.ascontiguousarray(np.asarray(a, f32)[:n_layers]).astype(bf16_np)

    # [L, H, D, HS] -> [L, D, H*HS] (head-concat order preserved)
    wq_all = np.asarray(Wq, f32)[:n_layers].transpose(0, 2, 1, 3).reshape(
        n_layers, D, D) * (HS ** -0.5)
    wk_all = np.asarray(Wk, f32)[:n_layers].transpose(0, 2, 1, 3).reshape(
        n_layers, D, D)
    wv_all = np.asarray(Wv, f32)[:n_layers].transpose(0, 2, 1, 3).reshape(
        n_layers, D, D)
    wp_all = np.asarray(Wproj, f32)[:n_layers]          # [L, D, D] rows=heads
    w1 = to_bf(W1)
    w2 = to_bf(W2)
    wlm_pad = np.zeros((D, VPAD), f32)
    wlm_pad[:, :V] = np.asarray(Wlm, f32)
    wlm_bf = np.ascontiguousarray(wlm_pad.astype(bf16_np))

    assert not np.any(np.asarray(bproj)) and not np.any(np.asarray(b1)) \
        and not np.any(np.asarray(b2)), "kernel assumes zero biases"
    for _g in (ln1_g, ln2_g):
        assert np.all(np.asarray(_g) == 1.0), "kernel assumes LN gamma == 1"
    for _b in (ln1_b, ln2_b):
        assert not np.any(np.asarray(_b)), "kernel assumes LN beta == 0"
    assert np.all(np.asarray(lnf_g) == 1.0) and not np.any(np.asarray(lnf_b))

    in_maps = []
    for c in range(N_CORES):
        b = c % B
        half = c // B
        hsl = slice(half * HL * HS, (half + 1) * HL * HS)
        m = dict(
            x0=np.ascontiguousarray(x0_t[b][:, half * TM:(half + 1) * TM]),
            wq=np.ascontiguousarray(wq_all[:, :, hsl]).astype(bf16_np),
            wk=np.ascontiguousarray(wk_all[:, :, hsl]).astype(bf16_np),
            wv=np.ascontiguousarray(wv_all[:, :, hsl]).astype(bf16_np),
            wp=np.ascontiguousarray(wp_all[:, hsl, :]).astype(bf16_np),
            w1=w1, w2=w2, wlm=wlm_bf,
        )
        in_maps.append(m)
    return in_maps


def kernel(**inputs):
    nc = _get_nc()
    in_maps = _prep_in_maps(**inputs)
    res = run_bass_kernel_spmd(nc, in_maps, core_ids=list(range(N_CORES)))
    out = np.empty((B, T, V), np.float32)
    for b in range(B):
        out[b, :TM] = res.results[b]["logits"][:, :V].astype(np.float32)
        out[b, TM:] = res.results[b + B]["logits"][:, :V].astype(np.float32)
    return out


# revision 34
# speedup vs baseline: 1.0198x; 1.0198x over previous
"""Trainium2 Bass kernel for a 6-layer GPT forward pass (B=4, T=1024, D=512,
H=8, HS=64, FF=2048, V=50257) on 8 NeuronCores.

Strategy: Megatron-style tensor+sequence parallel PAIRS over batch.
  - Cores c and c+4 form a pair working on batch c%4.
  - The residual stream x, LayerNorms, and the MLP are sharded by TOKENS
    (core c owns tokens [512*(c//4), 512*(c//4)+512)).
  - QKV projections, attention, and the attn-out projection are sharded by
    HEADS (4 heads per core, host slices Wq/Wk/Wv/Wproj — data, not code,
    so the single SPMD NEFF stays uniform across cores).
  - Per layer: AllGather of the LN1 output h (bf16) so QKV sees all tokens,
    and a ReduceScatter(add) of the attn-projection partial update (fp32)
    back to token shards.  MLP needs no communication.
  - LM head: each core computes logits for ITS 512 tokens x the FULL vocab
    (bf16 out, host upcasts + reassembles).

All activations stay TRANSPOSED [D, tokens] so every matmul contracts on
partitions; per-token row stats are broadcast via the idle GPSIMD engine.
"""

import numpy as np
import ml_dtypes

import concourse.bass as bass
import concourse.bacc as bacc
import concourse.mybir as mybir
from concourse.bass import ts, ds
from concourse.tile import TileContext
from concourse.bass_utils import run_bass_kernel_spmd

# Prefer the combined ln+exp table set so Ln/Exp activations don't ping-pong
# ACT_TABLE_LOADs between per-function home sets (~1.3us per switch).
import concourse.hw_specs as _hw_specs
import concourse.bacc as _bacc_mod

_orig_get_tables = _hw_specs.get_activation_tables


def _tables_combined_first(module_arch):
    tabs = _orig_get_tables(module_arch)
    pref = "natural_log_exp_and_others"
    if pref not in tabs:
        return tabs
    excl = {AF.Exp, AF.Ln}
    return {k: (v if k == pref else (v - excl)) for k, v in tabs.items()}


AF = mybir.ActivationFunctionType
_bacc_mod.get_activation_tables = _tables_combined_first
F32 = mybir.dt.float32
BF16 = mybir.dt.bfloat16

P = 128
B, T, D, H, HS, FF, L, V = 4, 1024, 512, 8, 64, 2048, 6, 50257
DC = D // P            # 4 d-chunks
FC = FF // P           # 16 ff-chunks
NT = T // P            # 8 token chunks of 128
NJ = T // 512          # 2 token chunks of 512
TM = 512               # tokens owned per core
NTM = TM // P          # 4 owned token chunks
HL = 4                 # heads per core
QC = HL * HS // P      # 2 chunks of my-head q/k dims
NV = 25216
VPAD = 2 * NV          # 50432 padded vocab (full, per core)
EPS = 1e-5
N_CORES = 8
GROUPS = [[0, 4], [1, 5], [2, 6], [3, 7]]

bf16_np = ml_dtypes.bfloat16


# --------------------------------------------------------------------------
# device program
# --------------------------------------------------------------------------

def build_nc(n_layers=L, debug=False):
    nc = bacc.Bacc()

    # ---------------- I/O ----------------
    x0_d = nc.dram_tensor("x0", [D, TM], F32, kind="ExternalInput")
    wq_d = nc.dram_tensor("wq", [n_layers, D, HL * HS], BF16, kind="ExternalInput")
    wk_d = nc.dram_tensor("wk", [n_layers, D, HL * HS], BF16, kind="ExternalInput")
    wv_d = nc.dram_tensor("wv", [n_layers, D, HL * HS], BF16, kind="ExternalInput")
    wp_d = nc.dram_tensor("wp", [n_layers, HL * HS, D], BF16, kind="ExternalInput")
    w1_d = nc.dram_tensor("w1", [n_layers, D, FF], BF16, kind="ExternalInput")
    w2_d = nc.dram_tensor("w2", [n_layers, FF, D], BF16, kind="ExternalInput")
    wlm_d = nc.dram_tensor("wlm", [D, VPAD], BF16, kind="ExternalInput")
    out_d = nc.dram_tensor("logits", [TM, VPAD], BF16, kind="ExternalOutput")

    # ---------------- constants ----------------
    # causal masks for transposed scores [t_k (partition), t_q (free)]:
    # block (r) valid iff t_k_local + 128*r <= t_q_local (within a 512 chunk)
    mask_np = np.zeros((P, 2, 1024), dtype=bf16_np)
    for pair in range(2):
        for half in range(2):
            r = 2 * pair + half
            tk = np.arange(P)[:, None] + 128 * r
            tq = np.arange(512)[None, :]
            mask_np[:, pair, half * 512:(half + 1) * 512] = \
                (tk <= tq).astype(bf16_np)
    mask_c = nc.inline_tensor(mask_np, name="cmask")
    ones_f32_c = nc.inline_tensor(np.ones((P, 1), np.float32), name="ones_f")
    ones_bf_c = nc.inline_tensor(np.ones((P, 1), bf16_np), name="ones_b")

    with TileContext(nc) as tc:
        with tc.tile_pool(name="persist", bufs=1) as persist:
            # ---- persistent tiles (token-sharded unless noted) ----
            x_sb = persist.tile([P, DC, TM], F32)          # my residual x^T
            hm_sb = persist.tile([P, DC, TM], BF16)        # my LN output
            hf_sb = persist.tile([P, DC, T], BF16)         # gathered h (full T)
            q_sb = persist.tile([P, QC, T], BF16)          # my-heads Q^T
            k_sb = persist.tile([P, QC, T], BF16)          # my-heads K^T
            v_sb = persist.tile([P, NT, HL, HS + 1], BF16)
            ac_sb = persist.tile([P, QC, T], BF16)         # my-heads attn^T
            pu_sb = persist.tile([P, DC, T], BF16)         # proj partial
            pr_sb = persist.tile([P, DC, TM], BF16)        # reduced update
            mid_sb = persist.tile([P, FC, TM], BF16)       # MLP mid^T
            mask_sb = persist.tile([P, 2, 1024], BF16)
            ones_f = persist.tile([P, 1], F32)
            ones_b = persist.tile([P, 1], BF16)

            nc.gpsimd.dma_start(mask_sb[:], mask_c[:])
            nc.gpsimd.dma_start(ones_f[:], ones_f32_c[:])
            nc.gpsimd.dma_start(ones_b[:], ones_bf_c[:])
            nc.gpsimd.dma_start(
                x_sb[:], x0_d[:].rearrange("(c p) t -> p c t", p=P))

            nc.vector.memset(v_sb[:, :, :, HS], 1.0)

            with (
                tc.tile_pool(name="wqkv", bufs=1) as wqkv_pool,
                tc.tile_pool(name="w1p", bufs=1) as w1_pool,
                tc.tile_pool(name="w2p", bufs=1) as w2_pool,
                tc.tile_pool(name="tmp", bufs=2) as tmp_pool,
                tc.tile_pool(name="wei", bufs=4) as wei_pool,
                tc.tile_pool(name="rows", bufs=2) as row_pool,
                tc.tile_pool(name="dramcc", bufs=2, space="DRAM") as dram_pool,
                tc.tile_pool(name="ps_wide", bufs=2, space="PSUM") as ps_wide,
                tc.tile_pool(name="ps_att", bufs=4, space="PSUM") as ps_att,
            ):
                # ---- helpers ----
                def layer_norm(src_sb, dst_sb, n_tok=TM):
                    """src [P, DC, n_tok] f32 -> dst bf16; LN over D (my
                    tokens only).  gamma==1 / beta==0 (asserted host-side)."""
                    for j in range(n_tok // 512):
                        sl = ts(j, 512)
                        xsq = tmp_pool.tile([P, DC, 512], BF16, tag="xsq")
                        for c in range(DC):
                            nc.scalar.activation(
                                xsq[:, c, :], src_sb[:, c, sl], AF.Square)
                        st_s = ps_att.tile([1, 512], F32, tag="att")
                        st_q = ps_att.tile([1, 512], F32, tag="att")
                        for c in range(DC):
                            nc.tensor.matmul(st_s[:], ones_f[:],
                                             src_sb[:, c, sl],
                                             start=(c == 0), stop=(c == DC - 1))
                            nc.tensor.matmul(st_q[:], ones_b[:], xsq[:, c, :],
                                             start=(c == 0), stop=(c == DC - 1))
                        r_mun = row_pool.tile([1, 512], F32, tag="r_mun")
                        r_msq = row_pool.tile([1, 512], F32, tag="r_msq")
                        r_var = row_pool.tile([1, 512], F32, tag="r_var")
                        rstd_b = row_pool.tile([1, 512], BF16, tag="rstd_b")
                        nmr_b = row_pool.tile([1, 512], BF16, tag="nmr_b")
                        nc.vector.tensor_scalar_mul(r_mun[:], st_s[:], -1.0 / D)
                        nc.vector.tensor_scalar_mul(r_msq[:], st_q[:], 1.0 / D)
                        nc.vector.tensor_mul(r_var[:], r_mun[:], r_mun[:])
                        nc.vector.tensor_sub(r_var[:], r_msq[:], r_var[:])
                        nc.vector.tensor_scalar_add(r_var[:], r_var[:], EPS)
                        nc.scalar.activation(r_var[:], r_var[:], AF.Ln)
                        nc.scalar.activation(rstd_b[:], r_var[:], AF.Exp,
                                             scale=-0.5)
                        nc.vector.tensor_mul(nmr_b[:], r_mun[:], rstd_b[:])
                        bc = tmp_pool.tile([P, 2, 512], BF16, tag="lnbc")
                        nc.gpsimd.partition_broadcast(bc[:, 0, :], rstd_b[:])
                        nc.gpsimd.partition_broadcast(bc[:, 1, :], nmr_b[:])
                        for c in range(DC):
                            tmp = tmp_pool.tile([P, 512], F32, tag="lnt")
                            nc.vector.tensor_mul(tmp[:], src_sb[:, c, sl],
                                                 bc[:, 0, :])
                            nc.vector.tensor_add(dst_sb[:, c, sl], tmp[:],
                                                 bc[:, 1, :])

                def linear_T(w_sb, src_sb, M_chunks, K_chunks, n_tok, evict):
                    for j in range(n_tok // 512):
                        for m in range(M_chunks):
                            pt = ps_wide.tile([P, 512], F32, tag="wide")
                            for c in range(K_chunks):
                                nc.tensor.matmul(pt[:], w_sb[:, c, ts(m, P)],
                                                 src_sb[:, c, ts(j, 512)],
                                                 start=(c == 0),
                                                 stop=(c == K_chunks - 1))
                            evict(pt, m, j)

                # ================= transformer layers =================
                for l in range(n_layers):
                    wq_sb = wqkv_pool.tile([P, DC, HL * HS], BF16, tag="wq")
                    wk_sb = wqkv_pool.tile([P, DC, HL * HS], BF16, tag="wk")
                    wv_sb = wqkv_pool.tile([P, DC, HL * HS], BF16, tag="wv")
                    wp_sb = wqkv_pool.tile([P, QC, D], BF16, tag="wp")
                    w1_sb = w1_pool.tile([P, DC, FF], BF16, tag="w1")
                    w2_sb = w2_pool.tile([P, FC, D], BF16, tag="w2")
                    nc.gpsimd.dma_start(
                        wq_sb[:], wq_d[l].rearrange("(c p) m -> p c m", p=P))
                    nc.gpsimd.dma_start(
                        wk_sb[:], wk_d[l].rearrange("(c p) m -> p c m", p=P))
                    nc.gpsimd.dma_start(
                        wv_sb[:], wv_d[l].rearrange("(c p) m -> p c m", p=P))
                    nc.gpsimd.dma_start(
                        wp_sb[:], wp_d[l].rearrange("(c p) m -> p c m", p=P))
                    nc.gpsimd.dma_start(
                        w1_sb[:], w1_d[l].rearrange("(c p) m -> p c m", p=P))
                    nc.gpsimd.dma_start(
                        w2_sb[:], w2_d[l].rearrange("(c p) m -> p c m", p=P))

                    # -- LN1 (my tokens) --
                    layer_norm(x_sb, hm_sb)

                    # -- AllGather h in two half-token collectives so the
                    # second hides behind QKV matmuls on the first set.
                    # Set s covers absolute tokens {g*512+s*256..+256}.
                    for s in range(2):
                        ag_in = dram_pool.tile([P, DC, 256], BF16,
                                               tag=f"ag_in{s}")
                        ag_out = dram_pool.tile([2, P, DC, 256], BF16,
                                                tag=f"ag_out{s}")
                        nc.gpsimd.dma_start(
                            ag_in[:], hm_sb[:, :, ds(s * 256, 256)])
                        nc.gpsimd.collective_compute(
                            "AllGather", mybir.AluOpType.bypass,
                            replica_groups=GROUPS,
                            ins=[ag_in.opt()], outs=[ag_out.opt()])
                        for g in range(2):
                            nc.gpsimd.dma_start(
                                hf_sb[:, :, ds(g * 512 + s * 256, 256)],
                                ag_out[g])

                    # -- my-heads Q^T, K^T over all tokens (set-ordered) --
                    hf4 = hf_sb[:].rearrange("p c (g s n) -> p c g s n",
                                             g=2, s=2)
                    for s in range(2):
                        for dst, w_sb in ((q_sb, wq_sb), (k_sb, wk_sb)):
                            for m in range(QC):
                                pt = ps_wide.tile([P, 512], F32, tag="wide")
                                for c in range(DC):
                                    nc.tensor.matmul(
                                        pt[:], w_sb[:, c, ts(m, P)],
                                        hf4[:, c, :, s, :],
                                        start=(c == 0), stop=(c == DC - 1))
                                nc.vector.tensor_copy(
                                    dst[:, m, :].rearrange(
                                        "p (g s n) -> p g s n",
                                        g=2, s=2)[:, :, s, :],
                                    pt[:].rearrange("p (g n) -> p g n", g=2))

                        # -- V natural for this set's token chunks --
                        for tchunk in (s * 2, s * 2 + 1, 4 + s * 2,
                                       5 + s * 2):
                            pt = ps_wide.tile([P, 512], F32, tag="wide")
                            for c in range(DC):
                                nc.tensor.matmul(pt[:, :HL * HS],
                                                 hf_sb[:, c, ts(tchunk, P)],
                                                 wv_sb[:, c, :],
                                                 start=(c == 0),
                                                 stop=(c == DC - 1))
                            nc.vector.tensor_copy(
                                v_sb[:, tchunk, :, 0:HS],
                                pt[:, :HL * HS].rearrange(
                                    "p (h s) -> p h s", h=HL))

                    # -- attention: my 2 head-pairs, all tq --
                    for hp in range(HL // 2):
                        h0, h1 = 2 * hp, 2 * hp + 1
                        for j in range(NJ):
                            kmax = 4 * j + 4
                            pa0 = ps_att.tile([HS + 1, 512], F32, tag="att")
                            pa1 = ps_att.tile([HS + 1, 512], F32, tag="att")
                            for kp in range(kmax // 2):
                                kk0 = 2 * kp
                                r = kk0 - 4 * j
                                weis = []
                                for idx in (0, 1):
                                    off = 64 * idx
                                    pscr = ps_wide.tile([P, 1024], F32,
                                                        tag="wide")
                                    for half in (0, 1):
                                        nc.tensor.matmul(
                                            pscr[:, ds(half * 512, 512)],
                                            k_sb[off:off + HS, hp,
                                                 ts(kk0 + half, P)],
                                            q_sb[off:off + HS, hp,
                                                 ts(j, 512)],
                                            start=True, stop=True)
                                    wei = wei_pool.tile([P, 1024], BF16,
                                                        tag="wei")
                                    nc.scalar.activation(wei[:], pscr[:],
                                                         AF.Exp)
                                    if r >= 0:
                                        nc.vector.tensor_mul(
                                            wei[:], wei[:],
                                            mask_sb[:, r // 2, :])
                                    weis.append(wei)
                                for half in (0, 1):
                                    kk = kk0 + half
                                    hs_sl = ds(half * 512, 512)
                                    nc.tensor.matmul(
                                        pa0[:], v_sb[:, kk, h0, :],
                                        weis[0][:, hs_sl],
                                        start=(kk == 0),
                                        stop=(kk == kmax - 1))
                                    nc.tensor.matmul(
                                        pa1[:], v_sb[:, kk, h1, :],
                                        weis[1][:, hs_sl],
                                        start=(kk == 0),
                                        stop=(kk == kmax - 1))
                            for idx, pa in enumerate((pa0, pa1)):
                                off = 64 * idx
                                l_row = row_pool.tile([1, 512], BF16,
                                                      tag="l_row")
                                nc.vector.tensor_copy(l_row[:],
                                                      pa[HS:HS + 1, :])
                                lb = tmp_pool.tile([64, 512], BF16,
                                                   tag="lb")
                                nc.gpsimd.partition_broadcast(lb[:], l_row[:])
                                rinv = tmp_pool.tile([64, 512], F32,
                                                     tag="rinv")
                                nc.scalar.activation(rinv[:], lb[:], AF.Ln)
                                nc.scalar.activation(rinv[:], rinv[:], AF.Exp,
                                                     scale=-1.0)
                                nc.vector.tensor_mul(
                                    ac_sb[off:off + HS, hp, ts(j, 512)],
                                    pa[0:HS, :], rinv[:])

                    # -- proj partial (my heads), ReduceScatter to my tokens
                    linear_T(wp_sb, ac_sb, DC, QC, T,
                             lambda pt, m, j: nc.vector.tensor_copy(
                                 pu_sb[:, m, ts(j, 512)], pt[:]))
                    rs_in = dram_pool.tile([2, P, DC, 512], BF16, tag="rs_in")
                    rs_out = dram_pool.tile([P, DC, 512], BF16, tag="rs_out")
                    for g in range(2):
                        nc.gpsimd.dma_start(rs_in[g], pu_sb[:, :, ts(g, 512)])
                    nc.gpsimd.collective_compute(
                        "ReduceScatter", mybir.AluOpType.add,
                        replica_groups=GROUPS,
                        ins=[rs_in.opt()], outs=[rs_out.opt()])
                    nc.gpsimd.dma_start(pr_sb[:], rs_out[:])
                    for c in range(DC):
                        prf = tmp_pool.tile([P, 512], F32, tag="prf")
                        nc.vector.tensor_copy(prf[:], pr_sb[:, c, :])
                        nc.vector.tensor_add(x_sb[:, c, :], x_sb[:, c, :],
                                             prf[:])

                    # -- LN2 + MLP (my tokens, no comm) --
                    layer_norm(x_sb, hm_sb)

                    def evict_mid(pt, m, j):
                        nc.scalar.activation(mid_sb[:, m, ts(j, 512)], pt[:],
                                             AF.Relu)

                    def evict_resid(pt, m, j):
                        nc.vector.tensor_add(x_sb[:, m, ts(j, 512)],
                                             x_sb[:, m, ts(j, 512)], pt[:])

                    linear_T(w1_sb, hm_sb, FC, DC, TM, evict_mid)
                    linear_T(w2_sb, mid_sb, DC, FC, TM, evict_resid)

                # ================= final LN =================
                layer_norm(x_sb, hm_sb)

            # ================= logits (my tokens x full vocab) ========
            with (
                tc.tile_pool(name="wlmp", bufs=2) as wlm_pool,
                tc.tile_pool(name="stage", bufs=3) as stage_pool,
                tc.tile_pool(name="ps_log", bufs=6, space="PSUM") as ps_log,
            ):
                GW = 6 * 512
                n_groups = (VPAD + GW - 1) // GW
                for g in range(n_groups):
                    g0 = g * GW
                    gw = min(GW, VPAD - g0)
                    wlm_sb = wlm_pool.tile([P, DC, GW], BF16, tag="wlm")
                    nc.gpsimd.dma_start(
                        wlm_sb[:, :, :gw],
                        wlm_d[:][:, g0:g0 + gw].rearrange(
                            "(c p) n -> p c n", p=P))
                    n_sub = (gw + 511) // 512
                    for m in range(NTM):
                        st = stage_pool.tile([P, GW], BF16, tag="stage")
                        for n in range(n_sub):
                            nw = min(512, gw - n * 512)
                            pt = ps_log.tile([P, 512], F32, tag="log")
                            for c in range(DC):
                                nc.tensor.matmul(
                                    pt[:, :nw],
                                    hm_sb[:, c, ts(m, P)],
                                    wlm_sb[:, c, ds(n * 512, nw)],
                                    start=(c == 0), stop=(c == DC - 1))
                            if n % 2 == 0:
                                nc.scalar.copy(st[:, ds(n * 512, nw)],
                                               pt[:, :nw])
                            else:
                                nc.vector.tensor_copy(st[:, ds(n * 512, nw)],
                                                      pt[:, :nw])
                        nc.sync.dma_start(out_d[:][ts(m, P), g0:g0 + gw],
                                          st[:, :gw])

    nc.compile()
    return nc


# --------------------------------------------------------------------------
# host side
# --------------------------------------------------------------------------

_NC_CACHE = {}


def _get_nc(n_layers=L, debug=False):
    key = (n_layers, debug)
    if key not in _NC_CACHE:
        _NC_CACHE[key] = build_nc(n_layers, debug)
    return _NC_CACHE[key]


def _prep_in_maps(index, tok_emb, pos_emb, Wq, Wk, Wv, Wproj, bproj,
                  ln1_g, ln1_b, ln2_g, ln2_b, W1, b1, W2, b2,
                  lnf_g, lnf_b, Wlm, n_layers=L):
    f32 = np.float32
    idx = np.asarray(index)
    tok = np.asarray(tok_emb, f32)
    pos = np.asarray(pos_emb, f32)
    x0 = tok[idx] + pos[None, :T]                       # [B, T, D]
    x0_t = np.ascontiguousarray(x0.transpose(0, 2, 1))  # [B, D, T]

    def to_bf(a):
        return np.ascontiguousarray(np.asarray(a, f32)[:n_layers]).astype(bf16_np)

    # [L, H, D, HS] -> [L, D, H*HS] (head-concat order preserved)
    wq_all = np.asarray(Wq, f32)[:n_layers].transpose(0, 2, 1, 3).reshape(
        n_layers, D, D) * (HS ** -0.5)
    wk_all = np.asarray(Wk, f32)[:n_layers].transpose(0, 2, 1, 3).reshape(
        n_layers, D, D)
    wv_all = np.asarray(Wv, f32)[:n_layers].transpose(0, 2, 1, 3).reshape(
        n_layers, D, D)
    wp_all = np.asarray(Wproj, f32)[:n_layers]          # [L, D, D] rows=heads
    w1 = to_bf(W1)
    w2 = to_bf(W2)
    wlm_pad = np.zeros((D, VPAD), f32)
    wlm_pad[:, :V] = np.asarray(Wlm, f32)
    wlm_bf = np.ascontiguousarray(wlm_pad.astype(bf16_np))

    assert not np.any(np.asarray(bproj)) and not np.any(np.asarray(b1)) \
        and not np.any(np.asarray(b2)), "kernel assumes zero biases"
    for _g in (ln1_g, ln2_g):
        assert np.all(np.asarray(_g) == 1.0), "kernel assumes LN gamma == 1"
    for _b in (ln1_b, ln2_b):
        assert not np.any(np.asarray(_b)), "kernel assumes LN beta == 0"
    assert np.all(np.asarray(lnf_g) == 1.0) and not np.any(np.asarray(lnf_b))

    in_maps = []
    for c in range(N_CORES):
        b = c % B
        half = c // B
        hsl = slice(half * HL * HS, (half + 1) * HL * HS)
        m = dict(
            x0=np.ascontiguousarray(x0_t[b][:, half * TM:(half + 1) * TM]),
            wq=np.ascontiguousarray(wq_all[:, :, hsl]).astype(bf16_np),
            wk=np.ascontiguousarray(wk_all[:, :, hsl]).astype(bf16_np),
            wv=np.ascontiguousarray(wv_all[:, :, hsl]).astype(bf16_np),
            wp=np.ascontiguousarray(wp_all[:, hsl, :]).astype(bf16_np),
            w1=w1, w2=w2, wlm=wlm_bf,
        )
        in_maps.append(m)
    return in_maps


def kernel(**inputs):
    nc = _get_nc()
    in_maps = _prep_in_maps(**inputs)
    res = run_bass_kernel_spmd(nc, in_maps, core_ids=list(range(N_CORES)))
    out = np.empty((B, T, V), np.float32)
    for b in range(B):
        out[b, :TM] = res.results[b]["logits"][:, :V].astype(np.float32)
        out[b, TM:] = res.results[b + B]["logits"][:, :V].astype(np.float32)
    return out
